# revision 19
# baseline (speedup 1.0000x reference)
"""Pairwise distance screen (CellList) kernel for 8 Trainium2 NeuronCores.

Computes the masked dense [N, N] lower-triangular distance matrix:
  out[i, j] = sqrt(|c_i - c_j|^2)  if  j < i, both species valid, d2 <= cutoff^2
            = 0                    otherwise

Strategy (2D spatial bucketing + bilinear distance on the tensor engine):
  - Atoms are sorted into 8 x-strips of 768 (by x), y-sorted within each
    strip.  Core c owns strip c = 6 row-blocks of 128.  For each block b
    the host packs the exact candidate column set
      C_b = { j < 128(b+1) : x_j >= bxmin-cut, bymin-cut <= y_j <= bymax+cut }
    (block bounds over its real rows).  For any pair (i, j), i > j, within
    the cutoff, the block of i satisfies all three conditions for j, so
    every pair is covered; duplicates (diagonal-block pairs) scatter the
    same value twice, which is benign.
  - d2 is computed bilinearly on the PE: d2 = r2_j - 2*ci.cj + r2_i, with
    coordinates centered per slot (x, y at the block median, z at 20) and
    3-way bf16 split so every product is exact; the K=21 matmul
    accumulates -2*ci.cj + r2_j in fp32 PSUM (6 product terms per dim;
    r2_j is a 3-way split of the f64 value).  |d2 error| ~1e-4, so only
    O(1) pairs within ~1e-4 of the cutoff boundary can flip vs the
    reference mask (~5e-3 Frobenius budget vs the 2e-2 gate).
  - The packed column order is arbitrary: the host keeps a per-slot
    column map and scatters nonzero result entries to
    full[max(oi,oj), min(oi,oj)], dropping sentinel padding, dummy
    (species<0) atoms, and the diagonal.
  - DVE op ADDSEL adds r2_i ([P,1] per-partition) and selects
    relu(t) if t < nextafter(cut2) else 0 (relu guards sqrt(-eps) on the
    diagonal).  ACT computes sqrt -> bf16 output slabs.
  - Slots are emitted widest-first so the trailing slot drains quickly.
"""

import threading

import numpy as np

N = 6144
P = 128
NCORES = 8
BPC = 6  # row-blocks per core
NB = N // P  # 48
NSTRIP = 8
SS = N // NSTRIP  # 768 atoms per x-strip
MMW = 512  # matmul free-dim width (one PSUM bank)
K = 21  # 6 product terms per dim + 3 r2 rows

_lock = threading.Lock()
_cache: dict = {}


def _order(Wks):
    """Slot processing order: widest first (fast tail drain)."""
    return sorted(range(BPC), key=lambda k: (-Wks[k], k))


def _register_ops():
    """Register the fused DVE op at runtime (visible to table-gen)."""
    import concourse.dve_ops as dve_ops
    from concourse.dve_spec import (
        C0,
        C1,
        Spec,
        Src0,
        Zero,
        _has_src1,
        lower,
        relu,
        select,
    )
    from concourse.dve_uop import DveOpSpec

    def make(name, body, ref):
        for op in dve_ops.OPS:
            if op.name == name:
                return op
        spec = Spec(body=body, reference=ref)
        row = 1 + len(dve_ops.OPS)
        assert row < 0x20
        shas = {}
        for ver in ("v3", "v4"):
            uops = lower(spec, ver=ver)
            shas[ver] = DveOpSpec(
                name=name, opcode=row, uops=uops, rd1_en=_has_src1(spec)
            ).sha(ver)
        op = dve_ops.DveOp(name, spec, subdim=False, uops_sha=shas)
        dve_ops._SUB_OPCODE_FOR_NAME[name] = row
        dve_ops.OPS.append(op)
        dve_ops.CUSTOM_DVE_SPECS[name] = spec
        return op

    # t = in0 + s0 ; out = (t < s1) ? max(t, 0) : 0
    def addsel_ref(in0, in1, s0, s1, imm2):
        t = (in0.astype(np.float32) + s0).astype(np.float32)
        return np.where(t < s1, np.maximum(t, 0.0), 0.0).astype(np.float32)

    t = Src0 + C0
    addsel = make("ADDSEL_ANT", select(t < C1, relu(t), Zero), addsel_ref)
    return addsel


def _build_program(Wks):
    import concourse.bacc as bacc
    import concourse.mybir as mybir
    import concourse.tile as tile

    addsel = _register_ops()

    WCOLS = BPC * P  # 768 weight columns
    MTOT = sum(Wks)
    Wmax = max(Wks)
    order = _order(Wks)  # widest first; columns are packed in this order
    offs = {}
    off = 0
    for k in order:
        offs[k] = off
        off += Wks[k]

    nc = bacc.Bacc("TRN2", target_bir_lowering=False, debug=False, num_devices=NCORES)
    f32 = mybir.dt.float32
    bf16 = mybir.dt.bfloat16

    # weights [:, :WCOLS] ++ packed moving columns [:, WCOLS:]
    wtmov = nc.dram_tensor("wtmov", [K, WCOLS + MTOT], bf16, kind="ExternalInput")
    consts = nc.dram_tensor("consts", [P, 8], f32, kind="ExternalInput")
    out = nc.dram_tensor("out", [BPC * P, Wmax], bf16, kind="ExternalOutput")

    head = WCOLS + Wks[order[0]]  # weights + first-processed slot's columns

    with tile.TileContext(nc) as tc:
        with (
            tc.tile_pool(name="const", bufs=1) as cpool,
            tc.tile_pool(name="work", bufs=4) as wpool,
            tc.tile_pool(name="outp", bufs=4) as spool,
            tc.tile_pool(name="psx", bufs=3, space="PSUM") as ppx,
        ):
            wm_t = cpool.tile([K, WCOLS + MTOT], bf16, tag="wtmov")
            c_t = cpool.tile([P, 8], f32, tag="consts")
            warm_t = cpool.tile([P, 2], f32, tag="warm")

            # pull the ACT sqrt table in immediately (no DMA deps)
            nc.vector.memset(warm_t[:, 0:1], 1.0)
            nc.scalar.sqrt(warm_t[:, 1:2], warm_t[:, 0:1])

            nc.sync.dma_start(wm_t[:, 0:head], wtmov[:, 0:head])
            nc.sync.dma_start(wm_t[:, head:], wtmov[:, head:])
            nc.scalar.dma_start(c_t[:], consts[:])

            for k in order:
                Wk = Wks[k]
                o = WCOLS + offs[k]
                xb = ppx.tile([P, Wk], f32, tag="xb")
                for h in range(0, Wk, MMW):
                    hw = min(MMW, Wk - h)
                    nc.tensor.matmul(
                        xb[:, h : h + hw],
                        wm_t[:, k * P : (k + 1) * P],
                        wm_t[:, o + h : o + h + hw],
                        start=True,
                        stop=True,
                    )
                v = wpool.tile([P, Wk], f32, tag="v")
                nc.vector._custom_dve(
                    addsel,
                    out=v[:],
                    in0=xb[:],
                    s0=c_t[:, k : k + 1],
                    s1=c_t[:, 6:7],
                )
                s = spool.tile([P, Wk], bf16, tag="s")
                nc.scalar.sqrt(s[:], v[:])
                nc.sync.dma_start(out[k * P : (k + 1) * P, 0:Wk], s[:])

    nc.compile()
    return nc


def _get_program(Wks):
    with _lock:
        key = ("nc", Wks)
        if key not in _cache:
            _cache[key] = _build_program(Wks)
    return _cache[key]


def _split3_bf16_f64(v64: np.ndarray):
    """3-way bf16 split of float64 values: h+m+l captures ~24 mantissa bits."""
    import ml_dtypes

    bf = ml_dtypes.bfloat16
    h = v64.astype(bf)
    r1 = v64 - h.astype(np.float64)
    m = r1.astype(bf)
    r2 = r1 - m.astype(np.float64)
    l = r2.astype(bf)
    return h, m, l


def _mov21(xc, yc, zc, r2):
    rows = []
    for d in (xc, yc, zc):
        h, m, l = _split3_bf16_f64(d)
        rows += [h, m, l, h, m, h]
    ra, rb, rc = _split3_bf16_f64(r2)
    rows += [ra, rb, rc]
    return rows


def _prepare_inputs(species, coordinates, cutoff):
    import ml_dtypes

    bf = ml_dtypes.bfloat16
    coords = np.asarray(coordinates, dtype=np.float32).reshape(-1, 3).copy()
    assert coords.shape[0] == N
    valid = np.asarray(species).reshape(-1) >= 0
    if not valid.all():
        bad = np.where(~valid)[0]
        coords[bad] = (1.0e5 + 1.0e4 * np.arange(len(bad), dtype=np.float32))[:, None]

    cutf = float(np.float32(cutoff))
    x, y = coords[:, 0], coords[:, 1]
    p1 = np.argsort(x, kind="stable")
    perm = np.concatenate(
        [
            p1[s * SS : (s + 1) * SS][
                np.argsort(y[p1[s * SS : (s + 1) * SS]], kind="stable")
            ]
            for s in range(NSTRIP)
        ]
    )
    sx = coords[perm, 0].astype(np.float64)
    sy = coords[perm, 1].astype(np.float64)
    sz = coords[perm, 2].astype(np.float64)
    svalid = valid[perm]

    # exact candidate column sets per block
    cols = []
    sizes = np.zeros(NB, np.int64)
    for b in range(NB):
        rows = slice(P * b, P * b + P)
        rv = svalid[rows]
        if not rv.any():
            cols.append(np.empty(0, np.int64))
            continue
        rxv = sx[rows][rv]
        ryv = sy[rows][rv]
        bxmin = rxv.min()
        bymin, bymax = ryv.min(), ryv.max()
        j = np.arange(P * (b + 1))
        m = (sx[j] >= bxmin - cutf) & (sy[j] >= bymin - cutf) & (sy[j] <= bymax + cutf)
        cb = j[m]
        cols.append(cb)
        sizes[b] = len(cb)

    Wks = tuple(
        int(max(64, -(-int(sizes[k::BPC].max()) // 32) * 32)) for k in range(BPC)
    )
    assert max(Wks) <= 1024, f"slot width {max(Wks)} exceeds PSUM tile budget"
    MTOT = sum(Wks)
    WCOLS = BPC * P

    cut2 = np.float32(cutf) * np.float32(cutf)
    cut_hi = np.nextafter(cut2, np.float32(np.inf), dtype=np.float32)

    order = _order(Wks)
    offs = {}
    _off = 0
    for k in order:
        offs[k] = _off
        _off += Wks[k]

    in_maps = []
    colmaps = []
    for c in range(NCORES):
        wtmov = np.empty((K, WCOLS + MTOT), bf)
        consts = np.zeros((P, 8), np.float32)
        consts[:, 6] = cut_hi
        cmaps = []
        for k in range(BPC):
            b = BPC * c + k
            Wk = Wks[k]
            rows = slice(P * b, P * b + P)
            rv = svalid[rows]
            rx, ry, rz = sx[rows], sy[rows], sz[rows]
            cx = float(np.median(rx[rv])) if rv.any() else 0.0
            cy = float(np.median(ry[rv])) if rv.any() else 0.0
            cz = 20.0

            off = WCOLS + offs[k]
            cb = cols[b]
            ux = np.full(Wk, -1.0e4, np.float64)
            uy = np.zeros(Wk, np.float64)
            uz = np.zeros(Wk, np.float64)
            ux[: len(cb)] = sx[cb]
            uy[: len(cb)] = sy[cb]
            uz[: len(cb)] = sz[cb]
            ux -= cx
            uy -= cy
            uz -= cz
            for r, row in enumerate(_mov21(ux, uy, uz, ux * ux + uy * uy + uz * uz)):
                wtmov[r, off : off + Wk] = row

            wx, wy, wz = rx - cx, ry - cy, rz - cz
            wr2 = wx * wx + wy * wy + wz * wz
            wrows = []
            for d in (wx, wy, wz):
                h, m, l = _split3_bf16_f64(d)
                n2 = np.float32(-2.0)
                h2 = (n2 * h.astype(np.float32)).astype(bf)
                m2 = (n2 * m.astype(np.float32)).astype(bf)
                l2 = (n2 * l.astype(np.float32)).astype(bf)
                wrows += [h2, h2, h2, m2, m2, l2]
            ones = np.ones(P, bf)
            wrows += [ones, ones, ones]
            for r in range(K):
                wtmov[r, k * P : (k + 1) * P] = wrows[r]
            consts[:, k] = wr2.astype(np.float32)

            cmap = np.full(Wk, -1, np.int64)
            cmap[: len(cb)] = cb
            cmaps.append(cmap)
        in_maps.append({"wtmov": wtmov, "consts": consts})
        colmaps.append(cmaps)

    _cache["meta"] = (perm, Wks, svalid, colmaps)
    return in_maps


def _run(in_maps, trace=False):
    from concourse import bass_utils

    nc = _get_program(_cache["meta"][1])
    return bass_utils.run_bass_kernel_spmd(
        nc, in_maps, core_ids=list(range(NCORES)), trace=trace
    )


def _assemble(results, perm, Wks, svalid, colmaps):
    full = np.zeros((N, N), np.float32)
    for c in range(NCORES):
        slab = np.asarray(results[c]["out"]).astype(np.float32)
        for k in range(BPC):
            b = BPC * c + k
            vals = slab[k * P : (k + 1) * P, 0 : Wks[k]]
            rr, cc = np.nonzero(vals)
            if rr.size == 0:
                continue
            si = P * b + rr
            sj = colmaps[c][k][cc]
            keep = (sj >= 0) & svalid[si] & (si != sj)
            keep &= svalid[np.maximum(sj, 0)]
            if not keep.all():
                rr, cc, si, sj = rr[keep], cc[keep], si[keep], sj[keep]
            oi = perm[si]
            oj = perm[sj]
            hi = np.maximum(oi, oj)
            lo = np.minimum(oi, oj)
            full[hi, lo] = vals[rr, cc]
    return full


def kernel(species, coordinates, cutoff):
    in_maps = _prepare_inputs(species, coordinates, cutoff)
    res = _run(in_maps)
    perm, Wks, svalid, colmaps = _cache["meta"]
    return _assemble(res.results, perm, Wks, svalid, colmaps)


# revision 21
# speedup vs baseline: 1.0286x; 1.0286x over previous
"""Pairwise distance screen (CellList) kernel for 8 Trainium2 NeuronCores.

Computes the masked dense [N, N] lower-triangular distance matrix:
  out[i, j] = sqrt(|c_i - c_j|^2)  if  j < i, both species valid, d2 <= cutoff^2
            = 0                    otherwise

Strategy (2D spatial bucketing + bilinear distance on the tensor engine):
  - Atoms are sorted into 8 x-strips of 768 (by x), y-sorted within each
    strip.  Core c owns strip c = 6 row-blocks of 128.  For each block b
    the host packs the exact candidate column set
      C_b = { j < 128(b+1) : x_j >= bxmin-cut, bymin-cut <= y_j <= bymax+cut }
    (block bounds over its real rows).  For any pair (i, j), i > j, within
    the cutoff, the block of i satisfies all three conditions for j, so
    every pair is covered; duplicates (diagonal-block pairs) scatter the
    same value twice, which is benign.
  - d2 is computed bilinearly on the PE: d2 = r2_j - 2*ci.cj + r2_i, with
    coordinates centered per slot (x, y at the block median, z at 20) and
    3-way bf16 split so every product is exact; the K=21 matmul
    accumulates -2*ci.cj + r2_j in fp32 PSUM (6 product terms per dim;
    r2_j is a 3-way split of the f64 value).  |d2 error| ~1e-4, so only
    O(1) pairs within ~1e-4 of the cutoff boundary can flip vs the
    reference mask (~5e-3 Frobenius budget vs the 2e-2 gate).
  - The packed column order is arbitrary: the host keeps a per-slot
    column map and scatters nonzero result entries to
    full[max(oi,oj), min(oi,oj)], dropping sentinel padding, dummy
    (species<0) atoms, and the diagonal.
  - DVE op ADDSEL adds r2_i ([P,1] per-partition) and selects
    relu(t) if t < nextafter(cut2) else 0 (relu guards sqrt(-eps) on the
    diagonal).  ACT computes sqrt -> bf16 output slabs.
  - Slots are emitted widest-first so the trailing slot drains quickly.
"""

import threading

import numpy as np

N = 6144
P = 128
NCORES = 8
BPC = 6  # row-blocks per core
NB = N // P  # 48
NSTRIP = 8
SS = N // NSTRIP  # 768 atoms per x-strip
MMW = 512  # matmul free-dim width (one PSUM bank)
K = 21  # 6 product terms per dim + 3 r2 rows

_lock = threading.Lock()
_cache: dict = {}


def _order(Wks):
    """Slot processing order: widest first (fast tail drain)."""
    return sorted(range(BPC), key=lambda k: (-Wks[k], k))


def _register_ops():
    """Register the fused DVE op at runtime (visible to table-gen)."""
    import concourse.dve_ops as dve_ops
    from concourse.dve_spec import (
        C0,
        C1,
        Spec,
        Src0,
        Zero,
        _has_src1,
        lower,
        relu,
        select,
    )
    from concourse.dve_uop import DveOpSpec

    def make(name, body, ref):
        for op in dve_ops.OPS:
            if op.name == name:
                return op
        spec = Spec(body=body, reference=ref)
        row = 1 + len(dve_ops.OPS)
        assert row < 0x20
        shas = {}
        for ver in ("v3", "v4"):
            uops = lower(spec, ver=ver)
            shas[ver] = DveOpSpec(
                name=name, opcode=row, uops=uops, rd1_en=_has_src1(spec)
            ).sha(ver)
        op = dve_ops.DveOp(name, spec, subdim=False, uops_sha=shas)
        dve_ops._SUB_OPCODE_FOR_NAME[name] = row
        dve_ops.OPS.append(op)
        dve_ops.CUSTOM_DVE_SPECS[name] = spec
        return op

    # t = in0 + s0 ; out = (t < s1) ? max(t, 0) : 0
    def addsel_ref(in0, in1, s0, s1, imm2):
        t = (in0.astype(np.float32) + s0).astype(np.float32)
        return np.where(t < s1, np.maximum(t, 0.0), 0.0).astype(np.float32)

    t = Src0 + C0
    addsel = make("ADDSEL_ANT", select(t < C1, relu(t), Zero), addsel_ref)
    return addsel


def _build_program(Wks):
    import concourse.bacc as bacc
    import concourse.mybir as mybir
    import concourse.tile as tile

    addsel = _register_ops()

    WCOLS = BPC * P  # 768 weight columns
    MTOT = sum(Wks)
    Wmax = max(Wks)
    order = _order(Wks)  # widest first; columns are packed in this order
    offs = {}
    off = 0
    for k in order:
        offs[k] = off
        off += Wks[k]

    nc = bacc.Bacc("TRN2", target_bir_lowering=False, debug=False, num_devices=NCORES)
    f32 = mybir.dt.float32
    bf16 = mybir.dt.bfloat16

    # weights [:, :WCOLS] ++ packed moving columns [:, WCOLS:]
    wtmov = nc.dram_tensor("wtmov", [K, WCOLS + MTOT], bf16, kind="ExternalInput")
    consts = nc.dram_tensor("consts", [P, 8], f32, kind="ExternalInput")
    out = nc.dram_tensor("out", [BPC * P, Wmax], bf16, kind="ExternalOutput")

    # weights + the first two processed slots' columns land in DMA 1; the
    # rest rides behind and is consumed 2+ slots later
    head = WCOLS + Wks[order[0]] + Wks[order[1]]

    with tile.TileContext(nc) as tc:
        with (
            tc.tile_pool(name="const", bufs=1) as cpool,
            tc.tile_pool(name="work", bufs=4) as wpool,
            tc.tile_pool(name="outp", bufs=4) as spool,
            tc.tile_pool(name="psx", bufs=3, space="PSUM") as ppx,
        ):
            wm_t = cpool.tile([K, WCOLS + MTOT], bf16, tag="wtmov")
            c_t = cpool.tile([P, 8], f32, tag="consts")
            warm_t = cpool.tile([P, 2], f32, tag="warm")

            # pull the ACT sqrt table in immediately (no DMA deps)
            nc.vector.memset(warm_t[:, 0:1], 1.0)
            nc.scalar.sqrt(warm_t[:, 1:2], warm_t[:, 0:1])

            nc.sync.dma_start(wm_t[:, 0:head], wtmov[:, 0:head])
            nc.sync.dma_start(wm_t[:, head:], wtmov[:, head:])
            nc.scalar.dma_start(c_t[:], consts[:])

            for k in order:
                Wk = Wks[k]
                o = WCOLS + offs[k]
                xb = ppx.tile([P, Wk], f32, tag="xb")
                for h in range(0, Wk, MMW):
                    hw = min(MMW, Wk - h)
                    nc.tensor.matmul(
                        xb[:, h : h + hw],
                        wm_t[:, k * P : (k + 1) * P],
                        wm_t[:, o + h : o + h + hw],
                        start=True,
                        stop=True,
                    )
                v = wpool.tile([P, Wk], f32, tag="v")
                nc.vector._custom_dve(
                    addsel,
                    out=v[:],
                    in0=xb[:],
                    s0=c_t[:, k : k + 1],
                    s1=c_t[:, 6:7],
                )
                s = spool.tile([P, Wk], bf16, tag="s")
                nc.scalar.sqrt(s[:], v[:])
                nc.sync.dma_start(out[k * P : (k + 1) * P, 0:Wk], s[:])

    nc.compile()
    return nc


def _get_program(Wks):
    with _lock:
        key = ("nc", Wks)
        if key not in _cache:
            _cache[key] = _build_program(Wks)
    return _cache[key]


def _split3_bf16_f64(v64: np.ndarray):
    """3-way bf16 split of float64 values: h+m+l captures ~24 mantissa bits."""
    import ml_dtypes

    bf = ml_dtypes.bfloat16
    h = v64.astype(bf)
    r1 = v64 - h.astype(np.float64)
    m = r1.astype(bf)
    r2 = r1 - m.astype(np.float64)
    l = r2.astype(bf)
    return h, m, l


def _mov21(xc, yc, zc, r2):
    rows = []
    for d in (xc, yc, zc):
        h, m, l = _split3_bf16_f64(d)
        rows += [h, m, l, h, m, h]
    ra, rb, rc = _split3_bf16_f64(r2)
    rows += [ra, rb, rc]
    return rows


def _prepare_inputs(species, coordinates, cutoff):
    import ml_dtypes

    bf = ml_dtypes.bfloat16
    coords = np.asarray(coordinates, dtype=np.float32).reshape(-1, 3).copy()
    assert coords.shape[0] == N
    valid = np.asarray(species).reshape(-1) >= 0
    if not valid.all():
        bad = np.where(~valid)[0]
        coords[bad] = (1.0e5 + 1.0e4 * np.arange(len(bad), dtype=np.float32))[:, None]

    cutf = float(np.float32(cutoff))
    x, y = coords[:, 0], coords[:, 1]
    p1 = np.argsort(x, kind="stable")
    perm = np.concatenate(
        [
            p1[s * SS : (s + 1) * SS][
                np.argsort(y[p1[s * SS : (s + 1) * SS]], kind="stable")
            ]
            for s in range(NSTRIP)
        ]
    )
    sx = coords[perm, 0].astype(np.float64)
    sy = coords[perm, 1].astype(np.float64)
    sz = coords[perm, 2].astype(np.float64)
    svalid = valid[perm]

    # exact candidate column sets per block
    cols = []
    sizes = np.zeros(NB, np.int64)
    for b in range(NB):
        rows = slice(P * b, P * b + P)
        rv = svalid[rows]
        if not rv.any():
            cols.append(np.empty(0, np.int64))
            continue
        rxv = sx[rows][rv]
        ryv = sy[rows][rv]
        bxmin = rxv.min()
        bymin, bymax = ryv.min(), ryv.max()
        j = np.arange(P * (b + 1))
        m = (sx[j] >= bxmin - cutf) & (sy[j] >= bymin - cutf) & (sy[j] <= bymax + cutf)
        # same-strip columns have j < i => y_j <= y_i <= bymax (y-sorted)
        strip0 = SS * (b // BPC)
        m[strip0:] &= sy[strip0 : P * (b + 1)] <= bymax
        cb = j[m]
        cols.append(cb)
        sizes[b] = len(cb)

    Wks = tuple(
        int(max(64, -(-int(sizes[k::BPC].max()) // 32) * 32)) for k in range(BPC)
    )
    assert max(Wks) <= 1024, f"slot width {max(Wks)} exceeds PSUM tile budget"
    MTOT = sum(Wks)
    WCOLS = BPC * P

    cut2 = np.float32(cutf) * np.float32(cutf)
    cut_hi = np.nextafter(cut2, np.float32(np.inf), dtype=np.float32)

    order = _order(Wks)
    offs = {}
    _off = 0
    for k in order:
        offs[k] = _off
        _off += Wks[k]

    in_maps = []
    colmaps = []
    for c in range(NCORES):
        wtmov = np.empty((K, WCOLS + MTOT), bf)
        consts = np.zeros((P, 8), np.float32)
        consts[:, 6] = cut_hi
        cmaps = []
        for k in range(BPC):
            b = BPC * c + k
            Wk = Wks[k]
            rows = slice(P * b, P * b + P)
            rv = svalid[rows]
            rx, ry, rz = sx[rows], sy[rows], sz[rows]
            cx = float(np.median(rx[rv])) if rv.any() else 0.0
            cy = float(np.median(ry[rv])) if rv.any() else 0.0
            cz = 20.0

            off = WCOLS + offs[k]
            cb = cols[b]
            ux = np.full(Wk, -1.0e4, np.float64)
            uy = np.zeros(Wk, np.float64)
            uz = np.zeros(Wk, np.float64)
            ux[: len(cb)] = sx[cb]
            uy[: len(cb)] = sy[cb]
            uz[: len(cb)] = sz[cb]
            ux -= cx
            uy -= cy
            uz -= cz
            for r, row in enumerate(_mov21(ux, uy, uz, ux * ux + uy * uy + uz * uz)):
                wtmov[r, off : off + Wk] = row

            wx, wy, wz = rx - cx, ry - cy, rz - cz
            wr2 = wx * wx + wy * wy + wz * wz
            wrows = []
            for d in (wx, wy, wz):
                h, m, l = _split3_bf16_f64(d)
                n2 = np.float32(-2.0)
                h2 = (n2 * h.astype(np.float32)).astype(bf)
                m2 = (n2 * m.astype(np.float32)).astype(bf)
                l2 = (n2 * l.astype(np.float32)).astype(bf)
                wrows += [h2, h2, h2, m2, m2, l2]
            ones = np.ones(P, bf)
            wrows += [ones, ones, ones]
            for r in range(K):
                wtmov[r, k * P : (k + 1) * P] = wrows[r]
            consts[:, k] = wr2.astype(np.float32)

            cmap = np.full(Wk, -1, np.int64)
            cmap[: len(cb)] = cb
            cmaps.append(cmap)
        in_maps.append({"wtmov": wtmov, "consts": consts})
        colmaps.append(cmaps)

    _cache["meta"] = (perm, Wks, svalid, colmaps)
    return in_maps


def _run(in_maps, trace=False):
    from concourse import bass_utils

    nc = _get_program(_cache["meta"][1])
    return bass_utils.run_bass_kernel_spmd(
        nc, in_maps, core_ids=list(range(NCORES)), trace=trace
    )


def _assemble(results, perm, Wks, svalid, colmaps):
    full = np.zeros((N, N), np.float32)
    for c in range(NCORES):
        slab = np.asarray(results[c]["out"]).astype(np.float32)
        for k in range(BPC):
            b = BPC * c + k
            vals = slab[k * P : (k + 1) * P, 0 : Wks[k]]
            rr, cc = np.nonzero(vals)
            if rr.size == 0:
                continue
            si = P * b + rr
            sj = colmaps[c][k][cc]
            keep = (sj >= 0) & svalid[si] & (si != sj)
            keep &= svalid[np.maximum(sj, 0)]
            if not keep.all():
                rr, cc, si, sj = rr[keep], cc[keep], si[keep], sj[keep]
            oi = perm[si]
            oj = perm[sj]
            hi = np.maximum(oi, oj)
            lo = np.minimum(oi, oj)
            full[hi, lo] = vals[rr, cc]
    return full


def kernel(species, coordinates, cutoff):
    in_maps = _prepare_inputs(species, coordinates, cutoff)
    res = _run(in_maps)
    perm, Wks, svalid, colmaps = _cache["meta"]
    return _assemble(res.results, perm, Wks, svalid, colmaps)
